# revision 6
# baseline (speedup 1.0000x reference)
"""Multi-head attention kernel for Trainium2, head-parallel across 8 NeuronCores.

Math per head h (reference):
    scores  = X @ W[h] @ X.T / sqrt(D)          [N, N]
    weights = softmax(scores, axis=-1) + 1e-8
    out    += weights @ (X @ V[h])              [N, D], summed over heads

Sharding: H=40 heads split 5-per-core across 8 cores; X replicated.  Each core
computes the partial sum of its 5 heads' outputs; the host sums the 8 partials.

Per-core kernel layout (all matmuls contract over the partition axis):
    XT   [d, n]   = X^T               (PE transposes, done once)
    XV   [m, e]   = X @ V[h]          (natural layout, lhsT=XT tile)
    XWT  [e, n]   = W[h]^T "@" XT     (lhsT=W[h], rhs=XT)
    scT  [m, n]   = XT_tile^T @ XWT   (scores transposed: m on partitions)
    E    [m, n]   = exp(scT / sqrt(D))     (ACT, PSUM->SBUF)
    rs   [1, n]   = ones^T @ E        (softmax denominator via PE)
    rr   [1, n]   = 1 / rs            (DVE reciprocal)
    bc   [p, n]   = broadcast of rr   (K=1 matmul with ones row)
    avT  [e, n]   = XV_tile^T @ E     (unnormalised attention output, transposed)
    OUT  [e, n]  += avT * bc          (DVE), then PE-transpose to [n, e] at the end

The transposed-scores layout makes the AV contraction (over m) natural and the
softmax denominator is recovered with cheap M=1 / K=1 matmuls.

Matmul operands are stored as float16: full PE rate (1 cycle/row, like bf16)
with a 10-bit mantissa, and every operand here is comfortably inside fp16
range (X ~ N(0,1), exp values in [0.3, 3], XV ~ 0.1).  PSUM accumulation is
fp32 throughout.
"""

import numpy as np

N, D, H, NCORES = 2048, 128, 40, 8
HC = H // NCORES          # heads per core
NT = N // 128             # 128-row tiles of n/m
CH = N // 512             # 512-column chunks of n
SCALE = 1.0 / float(np.sqrt(np.float32(D)))

# Matmul operand dtype knob: "f16" (default), "bf16".
CFG = {"mm": "f16"}

_CACHE = {}


def _emit(ctx, tc, nc, X, W, V, out, cfg):
    from concourse import mybir
    from concourse.masks import make_identity

    f32 = mybir.dt.float32
    mdt = {"f16": mybir.dt.float16, "bf16": mybir.dt.bfloat16}[cfg["mm"]]
    Exp = mybir.ActivationFunctionType.Exp

    # ---- pools ----
    consts = ctx.enter_context(tc.tile_pool(name="consts", bufs=1))
    big = ctx.enter_context(tc.tile_pool(name="big", bufs=1))
    xwtp = ctx.enter_context(tc.tile_pool(name="xwtp", bufs=2))
    expp = ctx.enter_context(tc.tile_pool(name="expp", bufs=2))
    smallp = ctx.enter_context(tc.tile_pool(name="smallp", bufs=2))
    scp = ctx.enter_context(tc.tile_pool(name="scp", bufs=2, space="PSUM"))
    avp = ctx.enter_context(tc.tile_pool(name="avp", bufs=1, space="PSUM"))
    utilp = ctx.enter_context(tc.tile_pool(name="utilp", bufs=2, space="PSUM"))
    bcp = ctx.enter_context(tc.tile_pool(name="bcp", bufs=1, space="PSUM"))

    # ---- constants ----
    idt = consts.tile([128, 128], f32, tag="idt")
    make_identity(nc, idt[:])
    ones = consts.tile([128, 128], mdt, tag="ones")
    nc.gpsimd.memset(ones[:], 1.0)

    # ---- load X and transpose into XT [d, n] (stored in matmul dtype) ----
    X_stage = big.tile([128, N], f32, tag="xstage")
    for nt in range(NT):
        nc.sync.dma_start(out=X_stage[:, nt * 128:(nt + 1) * 128],
                          in_=X[nt * 128:(nt + 1) * 128, :])
    XT = big.tile([128, N], mdt, tag="xt")
    for nt in range(NT):
        pt = utilp.tile([128, 128], f32, tag="u")
        nc.tensor.transpose(pt[:], X_stage[:, nt * 128:(nt + 1) * 128], idt[:])
        nc.vector.tensor_copy(XT[:, nt * 128:(nt + 1) * 128], pt[:])

    # ---- load W, V and cast ----
    Wf = big.tile([128, HC * 128], f32, tag="wf")
    Vf = big.tile([128, HC * 128], f32, tag="vf")
    for h in range(HC):
        nc.sync.dma_start(out=Wf[:, h * 128:(h + 1) * 128], in_=W[h])
        nc.sync.dma_start(out=Vf[:, h * 128:(h + 1) * 128], in_=V[h])
    Wc = big.tile([128, HC * 128], mdt, tag="wc")
    Vc = big.tile([128, HC * 128], mdt, tag="vc")
    nc.vector.tensor_copy(Wc[:], Wf[:])
    nc.vector.tensor_copy(Vc[:], Vf[:])

    # ---- XV for all heads: XV[m, e], tiled [mt][128, HC*128] ----
    XV = big.tile([128, NT * HC * 128], mdt, tag="xv")
    for mt in range(NT):
        sct = scp.tile([128, 1024], f32, tag="sc")
        nc.tensor.matmul(sct[:, 0:512], XT[:, mt * 128:(mt + 1) * 128],
                         Vc[:, 0:512], start=True, stop=True)
        nc.tensor.matmul(sct[:, 512:512 + (HC - 4) * 128],
                         XT[:, mt * 128:(mt + 1) * 128],
                         Vc[:, 512:HC * 128], start=True, stop=True)
        nc.vector.tensor_copy(XV[:, mt * HC * 128:(mt + 1) * HC * 128],
                              sct[:, 0:HC * 128])

    OUT_acc = big.tile([128, N], f32, tag="oacc")

    for h in range(HC):
        # ---- XWT[e, n] for this head ----
        XWT = xwtp.tile([128, N], mdt, tag="xwt")
        for g in range(2):
            sct = scp.tile([128, 1024], f32, tag="sc")
            for j in range(2):
                c = 2 * g + j
                nc.tensor.matmul(sct[:, j * 512:(j + 1) * 512],
                                 Wc[:, h * 128:(h + 1) * 128],
                                 XT[:, c * 512:(c + 1) * 512],
                                 start=True, stop=True)
            nc.vector.tensor_copy(XWT[:, g * 1024:(g + 1) * 1024], sct[:, 0:1024])

        for c in range(CH):
            ncol = slice(c * 512, (c + 1) * 512)
            # scores (transposed) + exp
            EXP = expp.tile([128, NT * 512], mdt, tag="exp")
            for p in range(NT // 2):
                sct = scp.tile([128, 1024], f32, tag="sc")
                for j in range(2):
                    mt = 2 * p + j
                    nc.tensor.matmul(sct[:, j * 512:(j + 1) * 512],
                                     XT[:, mt * 128:(mt + 1) * 128],
                                     XWT[:, ncol],
                                     start=True, stop=True)
                nc.scalar.activation(EXP[:, p * 1024:(p + 1) * 1024],
                                     sct[:, 0:1024], Exp, scale=SCALE)
            # softmax denominator: rs[1, n] = sum_m exp
            RS = utilp.tile([1, 512], f32, tag="u")
            for mt in range(NT):
                nc.tensor.matmul(RS[:], ones[:, 0:1],
                                 EXP[:, mt * 512:(mt + 1) * 512],
                                 start=(mt == 0), stop=(mt == NT - 1))
            RSr = smallp.tile([1, 512], f32, tag="rsr")
            nc.vector.reciprocal(RSr[:], RS[:])
            RSh = smallp.tile([1, 512], mdt, tag="rsh")
            nc.vector.tensor_copy(RSh[:], RSr[:])
            # broadcast reciprocal across partitions via K=1 matmul
            BC = bcp.tile([128, 512], f32, tag="bc")
            nc.tensor.matmul(BC[:], ones[0:1, :], RSh[:], start=True, stop=True)
            BC_sb = smallp.tile([128, 512], f32, tag="bcsb")
            nc.vector.tensor_copy(BC_sb[:], BC[:])
            # AV (transposed): avT[e, n] accumulated over m tiles
            AV = avp.tile([128, 512], f32, tag="av")
            for mt in range(NT):
                nc.tensor.matmul(AV[:],
                                 XV[:, mt * HC * 128 + h * 128:
                                        mt * HC * 128 + (h + 1) * 128],
                                 EXP[:, mt * 512:(mt + 1) * 512],
                                 start=(mt == 0), stop=(mt == NT - 1))
            # normalise + accumulate over heads
            if h == 0:
                nc.vector.tensor_mul(OUT_acc[:, ncol], AV[:], BC_sb[:])
            else:
                tmp = smallp.tile([128, 512], f32, tag="tmp")
                nc.vector.tensor_mul(tmp[:], AV[:], BC_sb[:])
                nc.vector.tensor_add(OUT_acc[:, ncol], OUT_acc[:, ncol], tmp[:])

    # ---- transpose OUT_acc [e, n] -> out [n, e] and store ----
    for nt in range(NT):
        pt = utilp.tile([128, 128], f32, tag="u")
        nc.tensor.transpose(pt[:], OUT_acc[:, nt * 128:(nt + 1) * 128], idt[:])
        OUTN = smallp.tile([128, 128], f32, tag="outn")
        nc.vector.tensor_copy(OUTN[:], pt[:])
        nc.sync.dma_start(out=out[nt * 128:(nt + 1) * 128, :], in_=OUTN[:])


def build(num_devices=NCORES, cfg=None):
    import concourse.bacc as bacc
    import concourse.tile as tile
    from concourse import mybir
    from contextlib import ExitStack

    cfg = dict(CFG, **(cfg or {}))
    nc = bacc.Bacc("TRN2", target_bir_lowering=False, debug=False,
                   num_devices=num_devices)
    f32 = mybir.dt.float32
    X = nc.dram_tensor("X", [N, D], f32, kind="ExternalInput").ap()
    W = nc.dram_tensor("W", [HC, D, D], f32, kind="ExternalInput").ap()
    V = nc.dram_tensor("V", [HC, D, D], f32, kind="ExternalInput").ap()
    out = nc.dram_tensor("out", [N, D], f32, kind="ExternalOutput").ap()
    with tile.TileContext(nc) as tc:
        with ExitStack() as ctx:
            _emit(ctx, tc, nc, X, W, V, out, cfg)
    nc.compile()
    return nc


def _get_nc():
    key = tuple(sorted(CFG.items()))
    if key not in _CACHE:
        _CACHE[key] = build()
    return _CACHE[key]


def kernel(X, W, V):
    from concourse.bass_utils import run_bass_kernel_spmd

    X = np.ascontiguousarray(np.asarray(X, dtype=np.float32))
    W = np.ascontiguousarray(np.asarray(W, dtype=np.float32))
    V = np.ascontiguousarray(np.asarray(V, dtype=np.float32))
    nc = _get_nc()
    in_maps = [
        {"X": X,
         "W": np.ascontiguousarray(W[c * HC:(c + 1) * HC]),
         "V": np.ascontiguousarray(V[c * HC:(c + 1) * HC])}
        for c in range(NCORES)
    ]
    res = run_bass_kernel_spmd(nc, in_maps, list(range(NCORES)))
    partials = np.stack([res.results[c]["out"] for c in range(NCORES)])
    return partials.sum(axis=0, dtype=np.float32)
